# revision 22
# baseline (speedup 1.0000x reference)
"""Enframe (overlapping-frame unfold) kernel for Trainium2.

Math: out[b, c*FL + k, t] = x[b, c, t*HOP + k]  with FL=2048, HOP=512,
T = (S - FL)//HOP + 1 = 934.

Decomposition (k = 512*q + 128*i + p, q,i in [0,4), p in [0,128)):
    out[b, c*FL + 512q + 128i + p, t] = X[t+q, 128i+p]
where X[j, r] = x[b, c, j*512 + r] (j < 937). Per (b, c) this is one
937x512 -> 512x937 transpose; each of the 16 output row-blocks is a
contiguous column-slice XT[128i:128(i+1), q:q+934] written densely.

Schedule per core (one batch element per NeuronCore, 8-way data parallel):
  - HBM is the budget (~19.1 MB at ~390 GB/s aggregate over the three DMA
    dispatch rings). The kernel keeps every ring's FIFO non-empty from the
    first load to the last store so the HBM pipe never idles:
      * channel-0 load pieces are enqueued first, spread across all three
        rings (sync + scalar HWDGE, gpsimd SWDGE) so c0 lands ~5 us after
        the preamble; channel-1 pieces queue right behind them.
      * transposes (TensorE via identity matmul, PSUM) start as soon as c0
        is resident; DVE copies assemble each 128-row output block in SBUF.
      * each block's 4 dense ~478 KB store DMAs are assigned to rings by a
        greedy byte-balance so stores drain behind the remaining c1 loads
        and all three rings run dry together at the very end.
  - The NEFF's fixed ~6.3 us full-semaphore-file clear epilogue + barriers
    and the ~1.8 us framework preamble are invariant; everything between is
    paced by HBM.
"""

import numpy as np

import concourse.mybir as mybir
import concourse.tile as tile
from concourse import bacc, bass_utils

B, C, S = 8, 2, 480000
FL, HOP = 2048, 512
T = (S - FL) // HOP + 1          # 934 frames
NQ = FL // HOP                   # 4 hop-shifts per frame length
NJ = T + NQ - 1                  # 937 hop-chunks of input actually used
P = 128
NI = HOP // P                    # 4 row-blocks of 128 within a hop
NJC_FULL = NJ // P               # 7 full 128-row chunks
NJ_REM = NJ - NJC_FULL * P       # 41 remainder rows
F32 = mybir.dt.float32

_NC_CACHE = None


def _emit(tc, nc, x, ident_in, out):
    # x: [C, S] f32 (this core's batch element), out: [C*FL, T] f32
    # Ring split by descriptor size: the 2 KB-per-descriptor loads ride the
    # gpsimd SWDGE ring (Q7 packs up to 64 descriptors per packet, so small
    # descriptors still stream near line rate; HWDGE runs them ~30% slower),
    # while the 3736 B-per-descriptor stores alternate over the two HWDGE
    # rings (SP + Activation), which need no Q7 emission. Channel-0 load
    # pieces go first, split fine so transposes chase the load stream and
    # the first stores issue right after c0 lands; channel-1 queues behind.
    sy, sc, gp = nc.sync, nc.scalar, nc.gpsimd
    store_rr = [0]

    def store_dma(dst, src):
        eng = (sy, sc)[store_rr[0] & 1]
        store_rr[0] += 1
        eng.dma_start(dst, src)

    with tc.tile_pool(name="consts", bufs=1) as consts, \
         tc.tile_pool(name="loads", bufs=1) as loadp, \
         tc.tile_pool(name="xt", bufs=1) as xtp, \
         tc.tile_pool(name="ps", bufs=8, space="PSUM") as psp:
        ident = consts.tile([P, P], F32, name="ident")
        sy.dma_start(ident[:, :], ident_in[:, :])

        # Load layout: a_all[p, jc*HOP + r] = x[c, (jc*128 + p)*HOP + r]
        # (dense 2 KB rows per partition per jc chunk); a_rem holds the 41
        # leftover hop-chunks.
        a_alls, a_tails = [], []
        for c in range(C):
            a_alls.append(
                loadp.tile([P, NJC_FULL * HOP], F32, name=f"a{c}", tag=f"a{c}")
            )
            a_tails.append(
                loadp.tile([P, HOP], F32, name=f"at{c}", tag=f"at{c}")
            )
        # The 41 remainder hop-chunks load as a full 128-partition tail
        # tile (overlapping jc6 — the duplicate read is 0.26 MB and keeps
        # every load a clean 128-descriptor DMA; a 41-descriptor DMA gets
        # chopped onto ~2 SDMA engines and its semaphore then gates the
        # whole first block). Most of c0 rides the SWDGE ring in jc order;
        # two small single-piece DMAs go to the HWDGE rings early enough
        # to finish before the deep SWDGE queue starves them (SWDGE's
        # 64-descriptor packets monopolize shared engines). c1 streams
        # behind c0 on the SWDGE ring only, keeping the HWDGE rings clear
        # for c0's stores.
        JT0 = NJ - P                           # tail covers j in [JT0, NJ)
        avs, xv_fulls = [], []
        for c in range(C):
            xv_fulls.append(
                x[c, 0:NJC_FULL * P * HOP].rearrange(
                    "(jc p r) -> p jc r", p=P, r=HOP
                )
            )
            avs.append(a_alls[c][:, :].rearrange("p (jc r) -> p jc r", r=HOP))

        def load(c, j0, j1, eng):
            eng.dma_start(avs[c][:, j0:j1], xv_fulls[c][:, j0:j1])

        def load_tail(c, eng):
            xv = x[c, 0:NJ * HOP].rearrange("(j r) -> j r", r=HOP)
            eng.dma_start(a_tails[c][:, :], xv[JT0:NJ])

        load_tail(0, sy)
        load(0, 0, 2, gp)
        load(0, 2, 3, gp)
        load(0, 3, 4, sy)
        load(0, 4, 5, sy)
        load(0, 5, 6, sc)
        load(0, 6, 7, sc)
        load_tail(1, sc)
        load(1, 0, 4, gp)
        load(1, 4, 7, gp)

        # Transpose + store. xt tiles are distinct per (c, i) so no reuse
        # dependencies gate the pipeline; each block's 4 dense ~478 KB
        # stores enqueue the moment its 8 PSUM->SBUF copies land.
        # Per-block transpose order chases load-landing order (tail first,
        # then the gp pieces in jc order, then the sync/scalar pieces).
        jc_order = {
            0: (3, 4, 5, 6, NJC_FULL, 0, 1, 2),
            1: (NJC_FULL, *range(NJC_FULL)),
        }
        for c in range(C):
            a_all, a_tail = a_alls[c], a_tails[c]
            for i in range(NI):
                xt = xtp.tile([P, NJ], F32, name=f"xt{c}{i}", tag=f"xt{c}{i}")
                for jc in jc_order[c]:
                    pt = psp.tile([P, P], F32, name="pt", tag="pt")
                    if jc < NJC_FULL:
                        j0, nj = jc * P, P
                        src = a_all[:, jc * HOP + i * P: jc * HOP + (i + 1) * P]
                        nc.tensor.transpose(pt[:, :nj], src, ident[:nj, :nj])
                        nc.vector.tensor_copy(xt[:, j0:j0 + nj], pt[:, :nj])
                    else:
                        # Remainder: transpose a 64-row slice (matmul base
                        # partition must be 0/32/64) and copy out the 41
                        # columns that land past jc6.
                        j0 = NJC_FULL * P
                        off = j0 - (JT0 + 64)
                        src = a_tail[64:P, i * P:(i + 1) * P]
                        nc.tensor.transpose(pt[:, :64], src, ident[64:P, 64:P])
                        nc.vector.tensor_copy(
                            xt[:, j0:NJ], pt[:, off:off + NJ_REM]
                        )
                last_block = c == C - 1 and i == NI - 1
                for q in range(NQ):
                    base = c * FL + q * HOP + i * P
                    if last_block:
                        th = T // 2
                        store_dma(out[base:base + P, :th], xt[:, q:q + th])
                        store_dma(out[base:base + P, th:], xt[:, q + th:q + T])
                    else:
                        store_dma(out[base:base + P, :], xt[:, q:q + T])


def _build():
    nc = bacc.Bacc(
        "TRN2",
        target_bir_lowering=False,
        debug=False,
        enable_asserts=False,
        num_devices=B,
    )
    x = nc.dram_tensor("x", [C, S], F32, kind="ExternalInput").ap()
    ident_in = nc.dram_tensor("ident", [P, P], F32, kind="ExternalInput").ap()
    out = nc.dram_tensor("out", [C * FL, T], F32, kind="ExternalOutput").ap()
    with tile.TileContext(nc) as tc:
        _emit(tc, nc, x, ident_in, out)
    nc.compile()
    return nc


def _get_nc():
    global _NC_CACHE
    if _NC_CACHE is None:
        _NC_CACHE = _build()
    return _NC_CACHE


def make_in_maps(x):
    ident = np.eye(P, dtype=np.float32)
    return [
        {"x": np.ascontiguousarray(x[b]), "ident": ident} for b in range(B)
    ]


def kernel(**inputs):
    x = np.ascontiguousarray(np.asarray(inputs["x"]), dtype=np.float32)
    assert x.shape == (B, C, S), x.shape
    nc = _get_nc()
    res = bass_utils.run_bass_kernel_spmd(
        nc, make_in_maps(x), core_ids=list(range(B))
    )
    return np.stack([r["out"] for r in res.results], axis=0)
